# revision 15
# baseline (speedup 1.0000x reference)
"""Trainium2 Bass kernel for CMPNEncoder functional-group embedding (v5).

out = func_save_init + A @ W,  A[m,:] = sum_a count_m[a] * f_atoms[a,:].

Host prep: compact to the globally-referenced atom rows (~80% of all),
split them evenly across 8 cores, and merge each row's payload into ONE
p-major u8 stream: [133 bf16 feature bytes | 50 nibble-packed count
bytes] = 316 B/row, so every partition's chunk is a single contiguous
multi-KB DMA descriptor.

On device, per 128-row tile: vector/gpsimd unpack the count nibbles to
an fp8 e4m3 [128,100] tile (exact for counts <= 15, host-checked), the
tensor engine contracts counts^T @ features into a PSUM [100,133]
accumulator (fp8 lhsT x bf16 rhs), then A @ W runs on-device.  Host
sums the per-core [100,300] partials and adds func_save_init.
"""

import sys

sys.path.insert(0, "/opt/trn_rl_repo")

import ml_dtypes
import numpy as np

import concourse.bacc as bacc
import concourse.mybir as mybir
from concourse.bass_utils import run_bass_kernel_spmd
from concourse.tile import TileContext

N_ATOMS = 400_000
FDIM = 133
HID = 300
NSEG = 100
N_CORES = 8
CH = 16                                   # 128-row tiles per DMA chunk

FP8 = ml_dtypes.float8_e4m3fn
FBYTES = 2 * FDIM                         # 266 bf16 feature bytes per row


def _chunk_schedule(ntiles):
    """Uniform chunks; every chunk gets its own SBUF buffer (no recycle)."""
    sched, t0 = [], 0
    while t0 < ntiles:
        g = min(CH, ntiles - t0)
        sched.append((t0, g))
        t0 += g
    return sched


def build_nc(ntiles, nibble=True):
    f32, bf16 = mybir.dt.float32, mybir.dt.bfloat16
    u8, fp8 = mybir.dt.uint8, mybir.dt.float8e4
    rowb = FBYTES + (50 if nibble else NSEG)    # 316 or 366 B per row

    nc = bacc.Bacc("TRN2", target_bir_lowering=False, debug=False)

    mrg = nc.declare_dram_parameter("mrg", [128, ntiles, rowb], u8,
                                    isOutput=False)
    wmat = nc.declare_dram_parameter("wmat", [FDIM, HID], f32, isOutput=False)
    ident_d = nc.declare_dram_parameter("ident", [NSEG, NSEG], f32,
                                        isOutput=False)
    out_d = nc.declare_dram_parameter("out", [NSEG, HID], bf16, isOutput=True)

    sched = _chunk_schedule(ntiles)

    with TileContext(nc) as tc:
        with (
            tc.tile_pool(name="const", bufs=1) as cpool,
            tc.tile_pool(name="stream", bufs=len(sched)) as spool,
            tc.tile_pool(name="psA", bufs=1, space="PSUM") as psA,
            tc.tile_pool(name="psT", bufs=1, space="PSUM") as psT,
            tc.tile_pool(name="sb2", bufs=1) as sb2,
        ):
            # The full stream fits in SBUF: issue every chunk's DMA up
            # front so the 16 queues run saturated with no recycle deps.
            sts = []
            for t0, g in sched:
                st = spool.tile([128, CH, rowb], u8, tag="s")
                nc.sync.dma_start(out=st[:, 0:g, :], in_=mrg[:, t0:t0 + g, :])
                sts.append(st)

            ident_t = cpool.tile([NSEG, NSEG], f32, tag="ident")
            nc.sync.dma_start(out=ident_t[:, :], in_=ident_d[:, :])
            wa_t = cpool.tile([128, HID], f32, tag="wa")
            nc.sync.dma_start(out=wa_t[:, :], in_=wmat[0:128, :])
            wb_t = cpool.tile([FDIM - 128, HID], f32, tag="wb")
            nc.sync.dma_start(out=wb_t[:, :], in_=wmat[128:FDIM, :])

            # Split the PSUM accumulation: A1 covers all but the last two
            # chunks, A2 the rest.  A1's transpose + W-projection issue as
            # soon as A1 closes and hide under the tail of the stream.
            nsplit = len(sched) - 2 if len(sched) >= 4 else len(sched)
            n1 = sum(g for _, g in sched[:nsplit])
            a1_ps = psA.tile([NSEG, FDIM], f32, tag="A1")
            if nsplit < len(sched):
                a2_ps = psA.tile([NSEG, FDIM], f32, tag="A2")
            else:
                a2_ps = a1_ps
            o_ps = psT.tile([NSEG, HID], f32, tag="o")

            def proj(a_ps_, half, first, last):
                a_sb = sb2.tile([NSEG, FDIM], f32, tag=f"a_sb{half}")
                nc.vector.tensor_copy(out=a_sb[:, :], in_=a_ps_[:, :])
                t1 = psT.tile([128, NSEG], f32, tag=f"t1{half}")
                nc.tensor.transpose(out=t1[:, :], in_=a_sb[:, 0:128],
                                    identity=ident_t[:, :])
                t2 = psT.tile([FDIM - 128, NSEG], f32, tag=f"t2{half}")
                nc.tensor.transpose(out=t2[:, :], in_=a_sb[:, 128:FDIM],
                                    identity=ident_t[:, :])
                at1 = sb2.tile([128, NSEG], f32, tag=f"at1{half}")
                nc.vector.tensor_copy(out=at1[:, :], in_=t1[:, :])
                at2 = sb2.tile([FDIM - 128, NSEG], f32, tag=f"at2{half}")
                nc.scalar.activation(out=at2[:, :], in_=t2[:, :],
                                     func=mybir.ActivationFunctionType.Copy,
                                     bias=0.0, scale=1.0)
                nc.tensor.matmul(out=o_ps[:, :], lhsT=at1[:, :],
                                 rhs=wa_t[:, :], start=first, stop=False)
                nc.tensor.matmul(out=o_ps[:, :], lhsT=at2[:, :],
                                 rhs=wb_t[:, :], start=False, stop=last)

            tglob = 0
            for ck, (t0, g) in enumerate(sched):
                st = sts[ck]
                if ck == nsplit:
                    proj(a1_ps, 1, True, False)
                for j in range(g):
                    lhs = st[:, j, FBYTES:FBYTES + NSEG].bitcast(fp8)
                    a_ps = a1_ps if tglob < n1 else a2_ps
                    nc.tensor.matmul(
                        out=a_ps[:, :],
                        lhsT=lhs,
                        rhs=st[:, j, 0:FBYTES].bitcast(bf16),
                        start=(tglob in (0, n1)),
                        stop=(tglob in (n1 - 1, ntiles - 1)),
                    )
                    tglob += 1

            if nsplit < len(sched):
                proj(a2_ps, 2, False, True)
            else:
                proj(a1_ps, 1, True, True)

            o_sb = sb2.tile([NSEG, HID], bf16, tag="o_sb")
            nc.vector.tensor_copy(out=o_sb[:, :], in_=o_ps[:, :])
            nc.sync.dma_start(out=out_d[:, :], in_=o_sb[:, :])

    nc.compile()
    return nc


def prepare_inputs(f_atoms, W, func2atom, mapping, n_cores=N_CORES):
    flat = func2atom.astype(np.int64).ravel()
    seg = np.repeat(mapping.astype(np.int64), func2atom.shape[1])
    valid = flat > 0
    atom = flat[valid] - 1
    seg = seg[valid]

    ref_atoms, inv = np.unique(atom, return_inverse=True)
    nref = len(ref_atoms)
    ntiles = (nref + n_cores * 128 - 1) // (n_cores * 128)
    rows_pad = ntiles * 128                   # rows per core
    nrows = n_cores * rows_pad

    cnt = np.bincount(inv * NSEG + seg, minlength=nrows * NSEG)
    cnt = cnt.reshape(nrows, NSEG)
    nibble = False                # device-side nibble unpack too slow (DVE)

    payload = np.zeros((nrows, FBYTES + (50 if nibble else NSEG)),
                       dtype=np.uint8)
    payload[:nref, :FBYTES] = (
        f_atoms[ref_atoms].astype(ml_dtypes.bfloat16).view(np.uint8))
    if nibble:
        c8 = cnt.astype(np.uint8)
        payload[:, FBYTES:] = c8[:, :50] | (c8[:, 50:] << 4)
    else:
        payload[:, FBYTES:] = cnt.astype(np.float32).astype(FP8).view(np.uint8)

    ident = np.eye(NSEG, dtype=np.float32)
    wmat = W.astype(np.float32)
    in_maps = []
    for c in range(n_cores):
        sl = slice(c * rows_pad, (c + 1) * rows_pad)
        in_maps.append({
            "mrg": payload[sl].reshape(128, ntiles, -1),
            "wmat": wmat,
            "ident": ident,
        })
    return in_maps, ntiles, nibble


_CACHE = {}


def kernel(f_atoms, W, func2atom, mapping, func_save_init, _trace=False):
    in_maps, ntiles, nibble = prepare_inputs(f_atoms, W, func2atom, mapping)
    key = (ntiles, nibble)
    if key not in _CACHE:
        _CACHE[key] = build_nc(ntiles, nibble)
    nc = _CACHE[key]
    res = run_bass_kernel_spmd(nc, in_maps, list(range(N_CORES)),
                               trace=_trace)
    partial = sum(r["out"].astype(np.float32) for r in res.results)
    out = func_save_init.astype(np.float32) + partial
    if _trace:
        kernel.last_exec_time_ns = res.exec_time_ns
    return out


# revision 16
# speedup vs baseline: 1.0349x; 1.0349x over previous
"""Trainium2 Bass kernel for CMPNEncoder functional-group embedding (v5).

out = func_save_init + A @ W,  A[m,:] = sum_a count_m[a] * f_atoms[a,:].

Host prep: compact to the globally-referenced atom rows (~80% of all),
split them evenly across 8 cores, and merge each row's payload into ONE
p-major u8 stream: [133 bf16 feature bytes | 50 nibble-packed count
bytes] = 316 B/row, so every partition's chunk is a single contiguous
multi-KB DMA descriptor.

On device, per 128-row tile: vector/gpsimd unpack the count nibbles to
an fp8 e4m3 [128,100] tile (exact for counts <= 15, host-checked), the
tensor engine contracts counts^T @ features into a PSUM [100,133]
accumulator (fp8 lhsT x bf16 rhs), then A @ W runs on-device.  Host
sums the per-core [100,300] partials and adds func_save_init.
"""

import sys

sys.path.insert(0, "/opt/trn_rl_repo")

import ml_dtypes
import numpy as np

import concourse.bacc as bacc
import concourse.mybir as mybir
from concourse.bass_utils import run_bass_kernel_spmd
from concourse.tile import TileContext

N_ATOMS = 400_000
FDIM = 133
HID = 300
NSEG = 100
N_CORES = 8
CH = 8                                    # 128-row tiles per DMA chunk

FP8 = ml_dtypes.float8_e4m3fn
FBYTES = 2 * FDIM                         # 266 bf16 feature bytes per row


def _chunk_schedule(ntiles):
    """Uniform chunks; every chunk gets its own SBUF buffer (no recycle)."""
    sched, t0 = [], 0
    while t0 < ntiles:
        g = min(CH, ntiles - t0)
        sched.append((t0, g))
        t0 += g
    return sched


def build_nc(ntiles, nibble=True):
    f32, bf16 = mybir.dt.float32, mybir.dt.bfloat16
    u8, fp8 = mybir.dt.uint8, mybir.dt.float8e4
    rowb = FBYTES + (50 if nibble else NSEG)    # 316 or 366 B per row

    nc = bacc.Bacc("TRN2", target_bir_lowering=False, debug=False)

    mrg = nc.declare_dram_parameter("mrg", [128, ntiles, rowb], u8,
                                    isOutput=False)
    wmat = nc.declare_dram_parameter("wmat", [FDIM, HID], f32, isOutput=False)
    ident_d = nc.declare_dram_parameter("ident", [NSEG, NSEG], f32,
                                        isOutput=False)
    out_d = nc.declare_dram_parameter("out", [NSEG, HID], bf16, isOutput=True)

    sched = _chunk_schedule(ntiles)

    with TileContext(nc) as tc:
        with (
            tc.tile_pool(name="const", bufs=1) as cpool,
            tc.tile_pool(name="stream", bufs=len(sched)) as spool,
            tc.tile_pool(name="psA", bufs=1, space="PSUM") as psA,
            tc.tile_pool(name="psT", bufs=1, space="PSUM") as psT,
            tc.tile_pool(name="sb2", bufs=1) as sb2,
        ):
            # The full stream fits in SBUF: issue every chunk's DMA up
            # front so the 16 queues run saturated with no recycle deps.
            sts = []
            for t0, g in sched:
                st = spool.tile([128, CH, rowb], u8, tag="s")
                nc.sync.dma_start(out=st[:, 0:g, :], in_=mrg[:, t0:t0 + g, :])
                sts.append(st)

            ident_t = cpool.tile([NSEG, NSEG], f32, tag="ident")
            nc.sync.dma_start(out=ident_t[:, :], in_=ident_d[:, :])
            wa_t = cpool.tile([128, HID], f32, tag="wa")
            nc.sync.dma_start(out=wa_t[:, :], in_=wmat[0:128, :])
            wb_t = cpool.tile([FDIM - 128, HID], f32, tag="wb")
            nc.sync.dma_start(out=wb_t[:, :], in_=wmat[128:FDIM, :])

            # Split the PSUM accumulation: A1 covers all but the last two
            # chunks, A2 the rest.  A1's transpose + W-projection issue as
            # soon as A1 closes and hide under the tail of the stream.
            nsplit = len(sched) - 6 if len(sched) >= 10 else len(sched)
            n1 = sum(g for _, g in sched[:nsplit])
            a1_ps = psA.tile([NSEG, FDIM], f32, tag="A1")
            if nsplit < len(sched):
                a2_ps = psA.tile([NSEG, FDIM], f32, tag="A2")
            else:
                a2_ps = a1_ps
            o_ps = psT.tile([NSEG, HID], f32, tag="o")

            def proj(a_ps_, half, first, last):
                a_sb = sb2.tile([NSEG, FDIM], f32, tag=f"a_sb{half}")
                nc.vector.tensor_copy(out=a_sb[:, :], in_=a_ps_[:, :])
                t1 = psT.tile([128, NSEG], f32, tag=f"t1{half}")
                nc.tensor.transpose(out=t1[:, :], in_=a_sb[:, 0:128],
                                    identity=ident_t[:, :])
                t2 = psT.tile([FDIM - 128, NSEG], f32, tag=f"t2{half}")
                nc.tensor.transpose(out=t2[:, :], in_=a_sb[:, 128:FDIM],
                                    identity=ident_t[:, :])
                at1 = sb2.tile([128, NSEG], f32, tag=f"at1{half}")
                nc.vector.tensor_copy(out=at1[:, :], in_=t1[:, :])
                at2 = sb2.tile([FDIM - 128, NSEG], f32, tag=f"at2{half}")
                nc.scalar.activation(out=at2[:, :], in_=t2[:, :],
                                     func=mybir.ActivationFunctionType.Copy,
                                     bias=0.0, scale=1.0)
                nc.tensor.matmul(out=o_ps[:, :], lhsT=at1[:, :],
                                 rhs=wa_t[:, :], start=first, stop=False)
                nc.tensor.matmul(out=o_ps[:, :], lhsT=at2[:, :],
                                 rhs=wb_t[:, :], start=False, stop=last)

            tglob = 0
            for ck, (t0, g) in enumerate(sched):
                st = sts[ck]
                if ck == nsplit:
                    proj(a1_ps, 1, True, False)
                for j in range(g):
                    lhs = st[:, j, FBYTES:FBYTES + NSEG].bitcast(fp8)
                    a_ps = a1_ps if tglob < n1 else a2_ps
                    nc.tensor.matmul(
                        out=a_ps[:, :],
                        lhsT=lhs,
                        rhs=st[:, j, 0:FBYTES].bitcast(bf16),
                        start=(tglob in (0, n1)),
                        stop=(tglob in (n1 - 1, ntiles - 1)),
                    )
                    tglob += 1

            if nsplit < len(sched):
                proj(a2_ps, 2, False, True)
            else:
                proj(a1_ps, 1, True, True)

            o_sb = sb2.tile([NSEG, HID], bf16, tag="o_sb")
            nc.vector.tensor_copy(out=o_sb[:, :], in_=o_ps[:, :])
            nc.sync.dma_start(out=out_d[:, :], in_=o_sb[:, :])

    nc.compile()
    return nc


def prepare_inputs(f_atoms, W, func2atom, mapping, n_cores=N_CORES):
    flat = func2atom.astype(np.int64).ravel()
    seg = np.repeat(mapping.astype(np.int64), func2atom.shape[1])
    valid = flat > 0
    atom = flat[valid] - 1
    seg = seg[valid]

    ref_atoms, inv = np.unique(atom, return_inverse=True)
    nref = len(ref_atoms)
    ntiles = (nref + n_cores * 128 - 1) // (n_cores * 128)
    rows_pad = ntiles * 128                   # rows per core
    nrows = n_cores * rows_pad

    cnt = np.bincount(inv * NSEG + seg, minlength=nrows * NSEG)
    cnt = cnt.reshape(nrows, NSEG)
    nibble = False                # device-side nibble unpack too slow (DVE)

    payload = np.zeros((nrows, FBYTES + (50 if nibble else NSEG)),
                       dtype=np.uint8)
    payload[:nref, :FBYTES] = (
        f_atoms[ref_atoms].astype(ml_dtypes.bfloat16).view(np.uint8))
    if nibble:
        c8 = cnt.astype(np.uint8)
        payload[:, FBYTES:] = c8[:, :50] | (c8[:, 50:] << 4)
    else:
        payload[:, FBYTES:] = cnt.astype(np.float32).astype(FP8).view(np.uint8)

    ident = np.eye(NSEG, dtype=np.float32)
    wmat = W.astype(np.float32)
    in_maps = []
    for c in range(n_cores):
        sl = slice(c * rows_pad, (c + 1) * rows_pad)
        in_maps.append({
            "mrg": payload[sl].reshape(128, ntiles, -1),
            "wmat": wmat,
            "ident": ident,
        })
    return in_maps, ntiles, nibble


_CACHE = {}


def kernel(f_atoms, W, func2atom, mapping, func_save_init, _trace=False):
    in_maps, ntiles, nibble = prepare_inputs(f_atoms, W, func2atom, mapping)
    key = (ntiles, nibble)
    if key not in _CACHE:
        _CACHE[key] = build_nc(ntiles, nibble)
    nc = _CACHE[key]
    res = run_bass_kernel_spmd(nc, in_maps, list(range(N_CORES)),
                               trace=_trace)
    partial = sum(r["out"].astype(np.float32) for r in res.results)
    out = func_save_init.astype(np.float32) + partial
    if _trace:
        kernel.last_exec_time_ns = res.exec_time_ns
    return out


# revision 18
# speedup vs baseline: 1.0472x; 1.0119x over previous
"""Trainium2 Bass kernel for CMPNEncoder functional-group embedding (v7).

out = func_save_init + A @ W,  A[m,:] = sum_a count_m[a] * f_atoms[a,:].

Host prep: compact to the globally-referenced atom rows (~80% of all),
split them evenly across 8 cores, p-major layout.  Two DRAM streams per
core: bf16 feature rows (266 B/row) and nibble-packed count pairs
(50 B/row): byte (p, t, j) = cnt[2t, j] | cnt[2t+1, j] << 4.

Device: the full stream fits in SBUF, so every chunk's DMA issues up
front and the 16 queues run saturated end-to-end.  Per chunk the DVE
unpacks count nibbles with two contiguous u16 bitwise ANDs (0x0F0F ->
even-tile counts, 0xF0F0 -> 16x odd-tile counts) and, with the scalar
engine, casts u8 -> fp8 e4m3 (exact: counts <= 15, 16x counts <= 240).
The tensor engine accumulates even tiles into PSUM A_even and odd tiles
into A_odd16; the final unmix A = A_even + A_odd16/16 is one DVE op,
then A @ W runs on-device via two PE transposes.  Host sums the
per-core bf16 [100,300] partials and adds func_save_init.
"""

import sys

sys.path.insert(0, "/opt/trn_rl_repo")

import ml_dtypes
import numpy as np

import concourse.bacc as bacc
import concourse.mybir as mybir
from concourse.bass_utils import run_bass_kernel_spmd
from concourse.tile import TileContext

N_ATOMS = 400_000
FDIM = 133
HID = 300
NSEG = 100
N_CORES = 8
CH = 24                                   # 128-row tiles per DMA chunk

FP8 = ml_dtypes.float8_e4m3fn
FBYTES = 2 * FDIM                         # 266 bf16 feature bytes per row


def _chunk_schedule(ntiles):
    sched, t0 = [], 0
    while t0 < ntiles:
        g = min(CH, ntiles - t0)
        sched.append((t0, g))
        t0 += g
    return sched


def build_nc(ntiles, nibble=True):
    f32, bf16 = mybir.dt.float32, mybir.dt.bfloat16
    u8, u16, fp8 = mybir.dt.uint8, mybir.dt.uint16, mybir.dt.float8e4
    A = mybir.AluOpType
    assert ntiles % 2 == 0

    nc = bacc.Bacc("TRN2", target_bir_lowering=False, debug=False)

    feat = nc.declare_dram_parameter("feat", [128, ntiles, FBYTES], u8,
                                     isOutput=False)
    pk_rows = ntiles // 2 if nibble else ntiles
    pk_d = nc.declare_dram_parameter("pk", [128, pk_rows, NSEG], u8,
                                     isOutput=False)
    wmat = nc.declare_dram_parameter("wmat", [FDIM, HID], f32, isOutput=False)
    ident_d = nc.declare_dram_parameter("ident", [NSEG, NSEG], f32,
                                        isOutput=False)
    out_d = nc.declare_dram_parameter("out", [NSEG, HID], bf16, isOutput=True)

    sched = _chunk_schedule(ntiles)
    pch = CH // 2 if nibble else CH

    with TileContext(nc) as tc:
        with (
            tc.tile_pool(name="const", bufs=1) as cpool,
            tc.tile_pool(name="fstream", bufs=len(sched)) as fpool,
            tc.tile_pool(name="pstream", bufs=len(sched)) as ppool,
            tc.tile_pool(name="unp", bufs=6) as upool,
            tc.tile_pool(name="psA", bufs=1, space="PSUM") as psA,
            tc.tile_pool(name="psT", bufs=1, space="PSUM") as psT,
            tc.tile_pool(name="sb2", bufs=1) as sb2,
        ):
            # Whole stream fits in SBUF: all DMAs issue up front, queues
            # stay saturated with no pool-recycle dependencies.
            fts, pts = [], []
            for t0, g in sched:
                ft = fpool.tile([128, CH, FBYTES], u8, tag="f")
                nc.sync.dma_start(out=ft[:, 0:g, :], in_=feat[:, t0:t0 + g, :])
                fts.append(ft)
                pt = ppool.tile([128, pch, NSEG], u8, tag="p")
                if nibble:
                    nc.sync.dma_start(out=pt[:, 0:g // 2, :],
                                      in_=pk_d[:, t0 // 2:(t0 + g) // 2, :])
                else:
                    nc.sync.dma_start(out=pt[:, 0:g, :],
                                      in_=pk_d[:, t0:t0 + g, :])
                pts.append(pt)

            ident_t = cpool.tile([NSEG, NSEG], f32, tag="ident")
            nc.sync.dma_start(out=ident_t[:, :], in_=ident_d[:, :])
            wa_t = cpool.tile([128, HID], f32, tag="wa")
            nc.sync.dma_start(out=wa_t[:, :], in_=wmat[0:128, :])
            wb_t = cpool.tile([FDIM - 128, HID], f32, tag="wb")
            nc.sync.dma_start(out=wb_t[:, :], in_=wmat[128:FDIM, :])

            a_e = psA.tile([NSEG, FDIM], f32, tag="Ae")
            if nibble:
                a_o = psA.tile([NSEG, FDIM], f32, tag="Ao")
            o_ps = psT.tile([NSEG, HID], f32, tag="o")

            tglob = 0
            for ck, (t0, g) in enumerate(sched):
                ft, pt = fts[ck], pts[ck]
                if nibble:
                    gp = g // 2
                    tue = upool.tile([128, pch, NSEG], u8, tag="tue")
                    tuo = upool.tile([128, pch, NSEG], u8, tag="tuo")
                    nc.vector.tensor_scalar(
                        out=tue[:, 0:gp, :].bitcast(u16),
                        in0=pt[:, 0:gp, :].bitcast(u16),
                        scalar1=0x0F0F, scalar2=None, op0=A.bitwise_and)
                    nc.vector.tensor_scalar(
                        out=tuo[:, 0:gp, :].bitcast(u16),
                        in0=pt[:, 0:gp, :].bitcast(u16),
                        scalar1=0xF0F0, scalar2=None, op0=A.bitwise_and)
                    wue = upool.tile([128, pch, NSEG], fp8, tag="wue")
                    wuo = upool.tile([128, pch, NSEG], fp8, tag="wuo")
                    nc.scalar.activation(
                        out=wue[:, 0:gp, :], in_=tue[:, 0:gp, :],
                        func=mybir.ActivationFunctionType.Copy,
                        bias=0.0, scale=1.0)
                    nc.vector.tensor_scalar(
                        out=wuo[:, 0:gp, :], in0=tuo[:, 0:gp, :],
                        scalar1=0, scalar2=None, op0=A.add)
                for j in range(g):
                    if nibble:
                        even = (j % 2 == 0)
                        lhs = wue[:, j // 2, :] if even else wuo[:, j // 2, :]
                        tgt = a_e if even else a_o
                        start = tglob in (0, 1)
                        stop = tglob in (ntiles - 2, ntiles - 1)
                    else:
                        lhs = pt[:, j, :].bitcast(fp8)
                        tgt = a_e
                        start = tglob == 0
                        stop = tglob == ntiles - 1
                    nc.tensor.matmul(
                        out=tgt[:, :],
                        lhsT=lhs,
                        rhs=ft[:, j, :].bitcast(bf16),
                        start=start,
                        stop=stop,
                    )
                    tglob += 1

            a_sb = sb2.tile([NSEG, FDIM], f32, tag="a_sb")
            if nibble:
                ae_sb = sb2.tile([NSEG, FDIM], f32, tag="ae_sb")
                nc.scalar.activation(out=ae_sb[:, :], in_=a_e[:, :],
                                     func=mybir.ActivationFunctionType.Copy,
                                     bias=0.0, scale=1.0)
                nc.vector.scalar_tensor_tensor(
                    out=a_sb[:, :], in0=a_o[:, :], scalar=0.0625,
                    in1=ae_sb[:, :], op0=A.mult, op1=A.add)
            else:
                nc.vector.tensor_copy(out=a_sb[:, :], in_=a_e[:, :])
            t1 = psT.tile([128, NSEG], f32, tag="t1")
            nc.tensor.transpose(out=t1[:, :], in_=a_sb[:, 0:128],
                                identity=ident_t[:, :])
            t2 = psT.tile([FDIM - 128, NSEG], f32, tag="t2")
            nc.tensor.transpose(out=t2[:, :], in_=a_sb[:, 128:FDIM],
                                identity=ident_t[:, :])
            at1 = sb2.tile([128, NSEG], f32, tag="at1")
            nc.vector.tensor_copy(out=at1[:, :], in_=t1[:, :])
            at2 = sb2.tile([FDIM - 128, NSEG], f32, tag="at2")
            nc.scalar.activation(out=at2[:, :], in_=t2[:, :],
                                 func=mybir.ActivationFunctionType.Copy,
                                 bias=0.0, scale=1.0)
            nc.tensor.matmul(out=o_ps[:, :], lhsT=at1[:, :], rhs=wa_t[:, :],
                             start=True, stop=False)
            nc.tensor.matmul(out=o_ps[:, :], lhsT=at2[:, :], rhs=wb_t[:, :],
                             start=False, stop=True)
            o_sb = sb2.tile([NSEG, HID], bf16, tag="o_sb")
            nc.vector.tensor_copy(out=o_sb[:, :], in_=o_ps[:, :])
            nc.sync.dma_start(out=out_d[:, :], in_=o_sb[:, :])

    nc.compile()
    return nc


def prepare_inputs(f_atoms, W, func2atom, mapping, n_cores=N_CORES):
    flat = func2atom.astype(np.int64).ravel()
    seg = np.repeat(mapping.astype(np.int64), func2atom.shape[1])
    valid = flat > 0
    atom = flat[valid] - 1
    seg = seg[valid]

    ref_atoms, inv = np.unique(atom, return_inverse=True)
    nref = len(ref_atoms)
    ntiles = -(-nref // (n_cores * 256)) * 2      # even tile count per core
    rows_pad = ntiles * 128
    nrows = n_cores * rows_pad

    cnt = np.bincount(inv * NSEG + seg, minlength=nrows * NSEG)
    cnt = cnt.reshape(nrows, NSEG)
    nibble = bool(cnt.max() <= 15)

    feats = np.zeros((nrows, FBYTES), dtype=np.uint8)
    feats[:nref] = f_atoms[ref_atoms].astype(ml_dtypes.bfloat16).view(np.uint8)

    ident = np.eye(NSEG, dtype=np.float32)
    wmat = W.astype(np.float32)
    in_maps = []
    for c in range(n_cores):
        sl = slice(c * rows_pad, (c + 1) * rows_pad)
        c3 = cnt[sl].reshape(128, ntiles, NSEG)
        if nibble:
            c3 = c3.astype(np.uint8)
            pk = c3[:, 0::2, :] | (c3[:, 1::2, :] << 4)
        else:
            pk = c3.astype(np.float32).astype(FP8).view(np.uint8)
        in_maps.append({
            "feat": feats[sl].reshape(128, ntiles, FBYTES),
            "pk": np.ascontiguousarray(pk),
            "wmat": wmat,
            "ident": ident,
        })
    return in_maps, ntiles, nibble


_CACHE = {}


def kernel(f_atoms, W, func2atom, mapping, func_save_init, _trace=False):
    in_maps, ntiles, nibble = prepare_inputs(f_atoms, W, func2atom, mapping)
    key = (ntiles, nibble)
    if key not in _CACHE:
        _CACHE[key] = build_nc(ntiles, nibble)
    nc = _CACHE[key]
    res = run_bass_kernel_spmd(nc, in_maps, list(range(N_CORES)),
                               trace=_trace)
    partial = sum(r["out"].astype(np.float32) for r in res.results)
    out = func_save_init.astype(np.float32) + partial
    if _trace:
        kernel.last_exec_time_ns = res.exec_time_ns
    return out


# revision 19
# speedup vs baseline: 1.0960x; 1.0467x over previous
"""Trainium2 Bass kernel for CMPNEncoder functional-group embedding (v7.1).

out = func_save_init + A @ W,  A[m,:] = sum_a count_m[a] * f_atoms[a,:].

Host prep: compact to the globally-referenced atom rows (~80% of all),
split them evenly across 8 cores, p-major layout, tiles padded to a
multiple of CH.  One DRAM slab per chunk of CH 128-row tiles:
[CH x 266 B bf16 feature rows | CH/2 x 100 B nibble-packed count pairs]
so each chunk is ONE dma_start with ~7.6 KB contiguous per-partition
descriptors.  Packed byte (p, t, j) = cnt[2t, j] | cnt[2t+1, j] << 4.

Device: the full stream fits in SBUF, so every chunk's DMA issues up
front and the 16 queues run saturated end-to-end.  Per chunk the DVE
unpacks count nibbles with two contiguous u16 bitwise ANDs (0x0F0F ->
even-tile counts, 0xF0F0 -> 16x odd-tile counts) and, with the scalar
engine, casts u8 -> fp8 e4m3 (exact: counts <= 15, 16x counts <= 240).
The tensor engine accumulates even tiles into PSUM A_even and odd tiles
into A_odd16; the final unmix A = A_even + A_odd16/16 is one DVE op,
then A @ W runs on-device via two PE transposes.  Host sums the
per-core bf16 [100,300] partials and adds func_save_init.
"""

import sys

sys.path.insert(0, "/opt/trn_rl_repo")

import ml_dtypes
import numpy as np

import concourse.bacc as bacc
import concourse.mybir as mybir
from concourse.bass_utils import run_bass_kernel_spmd
from concourse.tile import TileContext

N_ATOMS = 400_000
FDIM = 133
HID = 300
NSEG = 100
N_CORES = 8
CH = 24                                   # 128-row tiles per DMA chunk

FP8 = ml_dtypes.float8_e4m3fn
FBYTES = 2 * FDIM                         # 266 bf16 feature bytes per row
FOFF = CH * FBYTES                        # slab offset of the count block


def build_nc(nch, nibble=True):
    f32, bf16 = mybir.dt.float32, mybir.dt.bfloat16
    u8, u16, fp8 = mybir.dt.uint8, mybir.dt.uint16, mybir.dt.float8e4
    A = mybir.AluOpType
    ntiles = nch * CH
    pkb = (CH // 2 if nibble else CH) * NSEG
    slab = FOFF + pkb

    nc = bacc.Bacc("TRN2", target_bir_lowering=False, debug=False)

    mrg = nc.declare_dram_parameter("mrg", [128, nch, slab], u8,
                                    isOutput=False)
    wmat = nc.declare_dram_parameter("wmat", [FDIM, HID], f32, isOutput=False)
    ident_d = nc.declare_dram_parameter("ident", [NSEG, NSEG], f32,
                                        isOutput=False)
    out_d = nc.declare_dram_parameter("out", [NSEG, HID], bf16, isOutput=True)

    with TileContext(nc) as tc:
        with (
            tc.tile_pool(name="const", bufs=1) as cpool,
            tc.tile_pool(name="stream", bufs=nch) as spool,
            tc.tile_pool(name="unp", bufs=6) as upool,
            tc.tile_pool(name="psA", bufs=1, space="PSUM") as psA,
            tc.tile_pool(name="psT", bufs=1, space="PSUM") as psT,
            tc.tile_pool(name="sb2", bufs=1) as sb2,
        ):
            # Whole stream fits in SBUF: all DMAs issue up front, queues
            # stay saturated with no pool-recycle dependencies.
            sts = []
            for ck in range(nch):
                st = spool.tile([128, slab], u8, tag="s")
                nc.sync.dma_start(out=st[:, :], in_=mrg[:, ck, :])
                sts.append(st)

            ident_t = cpool.tile([NSEG, NSEG], f32, tag="ident")
            nc.sync.dma_start(out=ident_t[:, :], in_=ident_d[:, :])
            wa_t = cpool.tile([128, HID], f32, tag="wa")
            nc.sync.dma_start(out=wa_t[:, :], in_=wmat[0:128, :])
            wb_t = cpool.tile([FDIM - 128, HID], f32, tag="wb")
            nc.sync.dma_start(out=wb_t[:, :], in_=wmat[128:FDIM, :])

            a_e = psA.tile([NSEG, FDIM], f32, tag="Ae")
            if nibble:
                a_o = psA.tile([NSEG, FDIM], f32, tag="Ao")
            o_ps = psT.tile([NSEG, HID], f32, tag="o")

            tglob = 0
            for ck in range(nch):
                st = sts[ck]
                pk = st[:, FOFF:FOFF + pkb]
                if nibble:
                    tue = upool.tile([128, pkb], u8, tag="tue")
                    tuo = upool.tile([128, pkb], u8, tag="tuo")
                    nc.vector.tensor_scalar(
                        out=tue[:, :].bitcast(u16), in0=pk.bitcast(u16),
                        scalar1=0x0F0F, scalar2=None, op0=A.bitwise_and)
                    nc.vector.tensor_scalar(
                        out=tuo[:, :].bitcast(u16), in0=pk.bitcast(u16),
                        scalar1=0xF0F0, scalar2=None, op0=A.bitwise_and)
                    wue = upool.tile([128, pkb], fp8, tag="wue")
                    wuo = upool.tile([128, pkb], fp8, tag="wuo")
                    nc.scalar.activation(
                        out=wue[:, :], in_=tue[:, :],
                        func=mybir.ActivationFunctionType.Copy,
                        bias=0.0, scale=1.0)
                    nc.vector.tensor_scalar(
                        out=wuo[:, :], in0=tuo[:, :],
                        scalar1=0, scalar2=None, op0=A.add)
                for j in range(CH):
                    rhs = st[:, j * FBYTES:(j + 1) * FBYTES].bitcast(bf16)
                    if nibble:
                        even = (j % 2 == 0)
                        i = j // 2
                        wu = wue if even else wuo
                        lhs = wu[:, i * NSEG:(i + 1) * NSEG]
                        tgt = a_e if even else a_o
                        start = tglob in (0, 1)
                        stop = tglob in (ntiles - 2, ntiles - 1)
                    else:
                        lhs = pk[:, j * NSEG:(j + 1) * NSEG].bitcast(fp8)
                        tgt = a_e
                        start = tglob == 0
                        stop = tglob == ntiles - 1
                    nc.tensor.matmul(out=tgt[:, :], lhsT=lhs, rhs=rhs,
                                     start=start, stop=stop)
                    tglob += 1

            a_sb = sb2.tile([NSEG, FDIM], f32, tag="a_sb")
            if nibble:
                ae_sb = sb2.tile([NSEG, FDIM], f32, tag="ae_sb")
                nc.scalar.activation(out=ae_sb[:, :], in_=a_e[:, :],
                                     func=mybir.ActivationFunctionType.Copy,
                                     bias=0.0, scale=1.0)
                nc.vector.scalar_tensor_tensor(
                    out=a_sb[:, :], in0=a_o[:, :], scalar=0.0625,
                    in1=ae_sb[:, :], op0=A.mult, op1=A.add)
            else:
                nc.vector.tensor_copy(out=a_sb[:, :], in_=a_e[:, :])
            t1 = psT.tile([128, NSEG], f32, tag="t1")
            nc.tensor.transpose(out=t1[:, :], in_=a_sb[:, 0:128],
                                identity=ident_t[:, :])
            t2 = psT.tile([FDIM - 128, NSEG], f32, tag="t2")
            nc.tensor.transpose(out=t2[:, :], in_=a_sb[:, 128:FDIM],
                                identity=ident_t[:, :])
            at1 = sb2.tile([128, NSEG], f32, tag="at1")
            nc.vector.tensor_copy(out=at1[:, :], in_=t1[:, :])
            at2 = sb2.tile([FDIM - 128, NSEG], f32, tag="at2")
            nc.scalar.activation(out=at2[:, :], in_=t2[:, :],
                                 func=mybir.ActivationFunctionType.Copy,
                                 bias=0.0, scale=1.0)
            nc.tensor.matmul(out=o_ps[:, :], lhsT=at1[:, :], rhs=wa_t[:, :],
                             start=True, stop=False)
            nc.tensor.matmul(out=o_ps[:, :], lhsT=at2[:, :], rhs=wb_t[:, :],
                             start=False, stop=True)
            o_sb = sb2.tile([NSEG, HID], bf16, tag="o_sb")
            nc.vector.tensor_copy(out=o_sb[:, :], in_=o_ps[:, :])
            nc.sync.dma_start(out=out_d[:, :], in_=o_sb[:, :])

    nc.compile()
    return nc


def prepare_inputs(f_atoms, W, func2atom, mapping, n_cores=N_CORES):
    flat = func2atom.astype(np.int64).ravel()
    seg = np.repeat(mapping.astype(np.int64), func2atom.shape[1])
    valid = flat > 0
    atom = flat[valid] - 1
    seg = seg[valid]

    ref_atoms, inv = np.unique(atom, return_inverse=True)
    nref = len(ref_atoms)
    nch = -(-nref // (n_cores * 128 * CH))     # chunks per core
    ntiles = nch * CH
    rows_pad = ntiles * 128
    nrows = n_cores * rows_pad

    cnt = np.bincount(inv * NSEG + seg, minlength=nrows * NSEG)
    cnt = cnt.reshape(nrows, NSEG)
    nibble = bool(cnt.max() <= 15)

    feats = np.zeros((nrows, FBYTES), dtype=np.uint8)
    feats[:nref] = f_atoms[ref_atoms].astype(ml_dtypes.bfloat16).view(np.uint8)

    ident = np.eye(NSEG, dtype=np.float32)
    wmat = W.astype(np.float32)
    in_maps = []
    for c in range(n_cores):
        sl = slice(c * rows_pad, (c + 1) * rows_pad)
        f4 = feats[sl].reshape(128, nch, CH * FBYTES)
        c3 = cnt[sl].reshape(128, ntiles, NSEG)
        if nibble:
            c3 = c3.astype(np.uint8)
            pk4 = (c3[:, 0::2, :] | (c3[:, 1::2, :] << 4)).reshape(
                128, nch, (CH // 2) * NSEG)
        else:
            pk4 = c3.astype(np.float32).astype(FP8).view(np.uint8).reshape(
                128, nch, CH * NSEG)
        in_maps.append({
            "mrg": np.ascontiguousarray(np.concatenate([f4, pk4], axis=2)),
            "wmat": wmat,
            "ident": ident,
        })
    return in_maps, nch, nibble


_CACHE = {}


def kernel(f_atoms, W, func2atom, mapping, func_save_init, _trace=False):
    in_maps, nch, nibble = prepare_inputs(f_atoms, W, func2atom, mapping)
    key = (nch, nibble)
    if key not in _CACHE:
        _CACHE[key] = build_nc(nch, nibble)
    nc = _CACHE[key]
    res = run_bass_kernel_spmd(nc, in_maps, list(range(N_CORES)),
                               trace=_trace)
    partial = sum(r["out"].astype(np.float32) for r in res.results)
    out = func_save_init.astype(np.float32) + partial
    if _trace:
        kernel.last_exec_time_ns = res.exec_time_ns
    return out
